# revision 49
# baseline (speedup 1.0000x reference)
"""Trainium2 Bass kernel for the BetaBernoulliMixture problem.

Math reformulation (no gammaln needed):
  post_mixweight = sigmoid(-(d + c0)),  c0 = log((1-w)/w), and the
  betaln-difference d = clog2 - clog1 telescopes into a per-row prefix
  sum along T:
    d[t]   = sum_{tau<t} ( ln(num2[tau]) - ln(den2[tau]) )
    num2   = num * (ab1 + tau),   den2 = den * (ab2 + tau)
    num    = obs ? a2 : b2,       den  = obs ? a1 : b1
    a_i    = alpha_i + s_prev,    b_i  = beta_i + f_prev
    ab_i   = alpha_i + beta_i
  with s_prev/f_prev the shifted cumulative success/failure counts.

v2 engine mapping (B=4096 rows split 512/core across 8 cores; rows on
SBUF partitions, T on the free dim, F=2048 t-chunks, rc-inner order):
  DVE : a1-scan, d-scan, den select (copy_predicated in place on the
        PSUM b1), small carry copies.
  PE  : u = ab1 + t into PSUM bank A (affine matmul from [ab1; ones]
        weights x [ones; iota] rows), b1 = u - a1 into PSUM bank B
        (same affine then accumulate (-I) @ a1).  fp32 matmuls.
  GPS : numP = obs*dd + den (stt), num2 = (numP + dbe) * u (stt),
        den2 = (iota + ab2t) * den (stt).
  ACT : a2 = a1 + dal (bf16 out), b2 = b1 + dbe (bf16), b1 -> bf16,
        Ln(num2), Ln(den2), post = Sigmoid(-d - c0) (bf16).  Uses two
        activation tables (ln/exp + sigmoid); posts are batched per
        t-chunk across the 4 row-chunks so the table switch cost is
        2 loads per t-chunk.
Outputs: a1 fp32, b1/a2/b2/post bf16 (host upcasts to fp32).
"""

import numpy as np

B, T = 4096, 8192
NCORES = 8
RPC = B // NCORES        # rows per core = 512
P = 128                  # SBUF partitions
RC_N = RPC // P          # row chunks per core = 4
F = 2048                 # t-chunk width
TC_N = T // F            # t chunks = 4
# al1, dal, dbe, dd, then (ab1+t0, ab2+t0) per chunk
NCONST = 4 + 2 * TC_N

_PROGRAM_CACHE = {}


def _patch_act_tables():
    """Restrict activation-table selection to natural_log_exp_and_others
    + sigmoid_and_others (keeps dict order so act_func_set_id stays valid)."""
    import concourse.bacc as bacc_mod
    import concourse.hw_specs as hw_specs
    if getattr(bacc_mod, "_act_tables_patched", False):
        return
    orig = hw_specs.get_activation_tables

    def filtered(arch):
        full = orig(arch)
        return {
            name: (funcs if name == "natural_log_exp_and_others" else set())
            for name, funcs in full.items()
        }

    bacc_mod.get_activation_tables = filtered
    bacc_mod._act_tables_patched = True


def _build_program(c0: float):
    import concourse.bacc as bacc
    import concourse.mybir as mybir
    from concourse.tile import TileContext

    _patch_act_tables()

    f32 = mybir.dt.float32
    bf16 = mybir.dt.bfloat16
    Alu = mybir.AluOpType
    Act = mybir.ActivationFunctionType

    nc = bacc.Bacc()
    obs_d = nc.dram_tensor("obs", [RPC, T], f32, kind="ExternalInput")
    rcst_d = nc.dram_tensor("rowconst", [RPC, NCONST], f32, kind="ExternalInput")
    wcst_d = nc.dram_tensor("wconst", [4, RPC], bf16, kind="ExternalInput")
    negi_d = nc.dram_tensor("negI", [P, P], f32, kind="ExternalInput")
    rmat_d = nc.dram_tensor("rmat", [4, T], bf16, kind="ExternalInput")
    a1_o = nc.dram_tensor("a1_out", [RPC, T], f32, kind="ExternalOutput")
    b1_o = nc.dram_tensor("b1_out", [RPC, T], f32, kind="ExternalOutput")
    a2_o = nc.dram_tensor("a2_out", [RPC, T], bf16, kind="ExternalOutput")
    b2_o = nc.dram_tensor("b2_out", [RPC, T], bf16, kind="ExternalOutput")
    pm_o = nc.dram_tensor("post_out", [RPC, T], bf16, kind="ExternalOutput")

    with TileContext(nc) as tc:
        with (
            tc.tile_pool(name="consts", bufs=1) as cpool,
            tc.tile_pool(name="a1p", bufs=2) as a1pool,
            tc.tile_pool(name="dp", bufs=2) as dpool,
            tc.tile_pool(name="work", bufs=2) as wpool,
            tc.tile_pool(name="psum", bufs=2, space="PSUM") as ppool,
        ):
            iota_t = cpool.tile([P, F], f32, tag="iota")
            nc.gpsimd.iota(
                iota_t[:], pattern=[[1, F]], base=0, channel_multiplier=0,
                allow_small_or_imprecise_dtypes=True,
            )

            c0n_t = cpool.tile([P, 1], f32, tag="c0n")
            nc.vector.memset(c0n_t[:], c0)
            negi_t = cpool.tile([P, P], f32, tag="negI")
            nc.sync.dma_start(negi_t[:], negi_d[:, :])
            wcst_t = cpool.tile([4, RPC], bf16, tag="wconst")
            nc.sync.dma_start(wcst_t[:], wcst_d[:, :])

            rows = []
            for rc in range(RC_N):
                r0 = rc * P
                rt = cpool.tile([P, NCONST], f32, tag=f"rows{rc}")
                nc.sync.dma_start(rt[:], rcst_d[r0:r0 + P, :])
                rows.append(rt)
            carry_a1 = [cpool.tile([P, 1], f32, tag=f"ca1_{rc}", name=f"ca1_{rc}")
                        for rc in range(RC_N)]
            carry_d = [cpool.tile([P, 1], f32, tag=f"cd_{rc}", name=f"cd_{rc}")
                       for rc in range(RC_N)]

            chunks = [(tci, rc) for tci in range(TC_N) for rc in range(RC_N)]
            NCH = len(chunks)
            obs_tiles = {}
            r_tiles = {}
            st1 = {}   # k -> (a1_t, b_ps, obs_t)
            st2 = {}   # k -> (num_t, den2_t)

            def fetch_obs(k):
                tci, rc = chunks[k]
                t0, r0 = tci * F, rc * P
                obs_t = wpool.tile([P, F], f32, tag="obs", bufs=3, name="obs")
                nc.sync.dma_start(obs_t[:], obs_d[r0:r0 + P, t0:t0 + F])
                obs_tiles[k] = obs_t

            def get_r(tci):
                if tci not in r_tiles:
                    t0 = tci * F
                    r_t = wpool.tile([4, F], bf16, tag="R", bufs=1, name="r_t")
                    nc.sync.dma_start(r_t[:], rmat_d[:, t0:t0 + F])
                    r_tiles[tci] = r_t
                return r_tiles[tci]

            def emit_x1(k):
                """obs prefetch, a1 scan, PE b1 matmuls."""
                tci, rc = chunks[k]
                r0 = rc * P
                if k + 1 < NCH:
                    fetch_obs(k + 1)
                obs_t = obs_tiles[k]
                rt = rows[rc]
                al1 = rt[:, 0:1]

                a1_t = a1pool.tile([P, F + 1], f32, tag="a1", bufs=3, name="a1")
                a1_init = al1 if tci == 0 else carry_a1[rc][:]
                nc.gpsimd.tensor_copy(a1_t[:, 0:1], a1_init)
                nc.vector.tensor_tensor_scan(
                    a1_t[:, 1:F + 1], obs_t[:], obs_t[:], a1_init,
                    Alu.add, Alu.bypass,
                )
                nc.vector.tensor_copy(carry_a1[rc][:], a1_t[:, F:F + 1])
                a1_v = a1_t[:, 0:F]

                # PE: b1 = (ab1 + t) - a1 into PSUM
                b_ps = ppool.tile([P, F], f32, tag="b1", name="b_ps")
                wsl = wcst_t[:, r0:r0 + P]
                r_t = get_r(tci)
                for q in range(F // 512):
                    sl = slice(q * 512, (q + 1) * 512)
                    rsl = r_t[:, q * 512:(q + 1) * 512]
                    nc.tensor.matmul(
                        b_ps[:, sl], wsl, rsl, start=True, stop=False)
                    nc.tensor.matmul(
                        b_ps[:, sl], negi_t[:], a1_v[:, sl],
                        start=False, stop=True)
                st1[k] = (a1_t, b_ps, obs_t)

            def emit_x2(k):
                """b1->SBUF, output casts + DMA, select, GPS num, den2."""
                tci, rc = chunks[k]
                t0, r0 = tci * F, rc * P
                a1_t, b_ps, obs_t = st1.pop(k)
                a1_v = a1_t[:, 0:F]
                rt = rows[rc]
                dal = rt[:, 1:2]
                dbe = rt[:, 2:3]
                dd = rt[:, 3:4]
                ab2t = rt[:, 5 + 2 * tci:6 + 2 * tci]

                b1_t = wpool.tile([P, F], f32, tag="b1sb", name="b1_t")
                nc.scalar.activation(b1_t[:], b_ps[:], Act.Identity)
                a2_t = wpool.tile([P, F], bf16, tag="a2", name="a2_t")
                nc.scalar.activation(a2_t[:], a1_v, Act.Identity, bias=dal)
                b2_t = wpool.tile([P, F], bf16, tag="b2", name="b2_t")
                nc.scalar.activation(b2_t[:], b1_t[:], Act.Identity, bias=dbe)
                nc.sync.dma_start(a1_o[r0:r0 + P, t0:t0 + F], a1_v)
                nc.sync.dma_start(b1_o[r0:r0 + P, t0:t0 + F], b1_t[:])
                nc.gpsimd.dma_start(a2_o[r0:r0 + P, t0:t0 + F], a2_t[:])
                nc.gpsimd.dma_start(b2_o[r0:r0 + P, t0:t0 + F], b2_t[:])

                # den = obs ? a1 : b1  (in place in SBUF)
                obs_mask = obs_t[:].bitcast(mybir.dt.uint32)
                nc.vector.copy_predicated(b1_t[:], obs_mask, a1_v)

                # seld = obs*dd + dbe (ACT); num = den + seld (GPS)
                # den2 = (iota + ab2t) * den (DVE stt)
                seld_t = wpool.tile([P, F], f32, tag="seld", bufs=1,
                                    name="seld_t")
                nc.scalar.activation(
                    seld_t[:], obs_t[:], Act.Identity, bias=dbe, scale=dd)
                num_t = wpool.tile([P, F], f32, tag="num", bufs=3, name="num_t")
                nc.gpsimd.tensor_tensor(
                    num_t[:], b1_t[:], seld_t[:], Alu.add)
                den2_t = wpool.tile([P, F], f32, tag="den2", bufs=3,
                                    name="den2_t")
                nc.vector.scalar_tensor_tensor(
                    den2_t[:], iota_t[:], ab2t, b1_t[:], Alu.add, Alu.mult)
                st2[k] = (num_t, den2_t)

            d_done = {}

            def emit_y(k):
                """num2 stt, logs, d-scan."""
                tci, rc = chunks[k]
                num_t, den2_t = st2.pop(k)
                rt = rows[rc]
                ab1t = rt[:, 4 + 2 * tci:5 + 2 * tci]
                num2_t = wpool.tile([P, F], f32, tag="num2", name="num2_t")
                nc.vector.scalar_tensor_tensor(
                    num2_t[:], iota_t[:], ab1t, num_t[:], Alu.add, Alu.mult)
                # logs in place
                nc.scalar.activation(num2_t[:], num2_t[:], Act.Ln)
                nc.scalar.activation(den2_t[:], den2_t[:], Act.Ln)

                # d: state = (lnum2 + state) - lden2, chained
                d_t = dpool.tile([P, F + 1], f32, tag="d", bufs=3, name="d_t")
                if tci == 0:
                    nc.gpsimd.memset(d_t[:, 0:1], 0.0)
                    d_init = 0.0
                else:
                    d_init = carry_d[rc][:]
                    nc.gpsimd.tensor_copy(d_t[:, 0:1], d_init)
                nc.vector.tensor_tensor_scan(
                    d_t[:, 1:F + 1], num2_t[:], den2_t[:], d_init,
                    Alu.add, Alu.subtract,
                )
                nc.vector.tensor_copy(carry_d[rc][:], d_t[:, F:F + 1])
                d_done[k] = d_t

            def emit_z(k):
                """post = exp(-ln(1 + exp(d + c0))) -- single act table."""
                tci, rc = chunks[k]
                t0, r0 = tci * F, rc * P
                d_t = d_done.pop(k)
                u_t = wpool.tile([P, F], f32, tag="postu", bufs=1, name="u_t")
                nc.scalar.activation(
                    u_t[:], d_t[:, 0:F], Act.Exp, bias=c0n_t[:])
                nc.scalar.activation(u_t[:], u_t[:], Act.Ln, bias=1.0)
                post_t = wpool.tile([P, F], bf16, tag="post", name="post_t")
                nc.scalar.activation(post_t[:], u_t[:], Act.Exp, scale=-1.0)
                nc.sync.dma_start(pm_o[r0:r0 + P, t0:t0 + F], post_t[:])

            # flat 4-stage software pipeline over all 16 chunks
            fetch_obs(0)
            for k in range(NCH + 3):
                if k < NCH:
                    emit_x1(k)
                if 1 <= k and k - 1 < NCH:
                    emit_x2(k - 1)
                if 2 <= k and k - 2 < NCH:
                    emit_y(k - 2)
                if 3 <= k:
                    emit_z(k - 3)
    nc.finalize()
    return nc


def _pack_inputs(alpha1, beta1, alpha2, beta2):
    """rowconst [B, NCONST]: al1, dal, dbe, dd, then ab2+t0 per chunk.
    wconst [2, B]: row0 = ab1, row1 = ones."""
    a1 = alpha1.astype(np.float32)
    b1 = beta1.astype(np.float32)
    a2 = alpha2.astype(np.float32)
    b2 = beta2.astype(np.float32)
    dal = a2 - a1
    dbe = b2 - b1
    cols = [a1, dal, dbe, dal - dbe]
    ab1 = a1 + b1
    ab2 = a2 + b2
    for tci in range(TC_N):
        cols.append(ab1 + np.float32(tci * F))
        cols.append(ab2 + np.float32(tci * F))
    rowconst = np.ascontiguousarray(np.stack(cols, axis=1), dtype=np.float32)
    import ml_dtypes
    bf = np.dtype(ml_dtypes.bfloat16)
    ab1_hi = (a1 + b1).astype(bf).astype(np.float32)
    ab1_lo = (a1 + b1) - ab1_hi
    ones = np.ones_like(ab1_hi)
    wconst = np.ascontiguousarray(
        np.stack([ab1_hi, ab1_lo, ones, ones], axis=0).astype(bf))
    return rowconst, wconst


def make_in_maps(obs_seq, alpha1, beta1, alpha2, beta2):
    obs_seq = np.ascontiguousarray(obs_seq, dtype=np.float32)
    rowconst, wconst = _pack_inputs(
        np.asarray(alpha1), np.asarray(beta1),
        np.asarray(alpha2), np.asarray(beta2),
    )
    import ml_dtypes
    bf = np.dtype(ml_dtypes.bfloat16)
    negI = np.ascontiguousarray(-np.eye(P, dtype=np.float32))
    g = np.arange(T, dtype=np.float32)
    g_hi = np.floor(g / 32.0) * 32.0
    g_lo = g - g_hi
    onesT = np.ones(T, np.float32)
    rmat = np.ascontiguousarray(
        np.stack([onesT, onesT, g_hi, g_lo], axis=0).astype(bf))
    in_maps = []
    for c in range(NCORES):
        r0 = c * RPC
        in_maps.append({
            "obs": obs_seq[r0:r0 + RPC],
            "rowconst": rowconst[r0:r0 + RPC],
            "wconst": np.ascontiguousarray(wconst[:, r0:r0 + RPC]),
            "negI": negI,
            "rmat": rmat,
        })
    return in_maps


def kernel(obs_seq, alpha1, beta1, alpha2, beta2, mixweight):
    from concourse.bass_utils import run_bass_kernel_spmd

    w = float(np.float32(mixweight))
    c0 = float(np.float32(np.log((1.0 - w) / w)))
    key = c0
    if key not in _PROGRAM_CACHE:
        _PROGRAM_CACHE[key] = _build_program(c0)
    nc = _PROGRAM_CACHE[key]

    in_maps = make_in_maps(obs_seq, alpha1, beta1, alpha2, beta2)
    res = run_bass_kernel_spmd(nc, in_maps, core_ids=list(range(NCORES)))
    out = np.empty((5, B, T), np.float32)
    names = ["a1_out", "b1_out", "a2_out", "b2_out", "post_out"]
    for c in range(NCORES):
        r0 = c * RPC
        for k, name in enumerate(names):
            out[k, r0:r0 + RPC] = np.asarray(res.results[c][name]).astype(np.float32)
    return out


# revision 57
# speedup vs baseline: 1.0390x; 1.0390x over previous
"""Trainium2 Bass kernel for the BetaBernoulliMixture problem.

Math reformulation (no gammaln needed):
  post_mixweight = sigmoid(-(d + c0)),  c0 = log((1-w)/w), and the
  betaln-difference d = clog2 - clog1 telescopes into a per-row prefix
  sum along T:
    d[t]   = sum_{tau<t} ( ln(num2[tau]) - ln(den2[tau]) )
    num2   = num * (ab1 + tau),   den2 = den * (ab2 + tau)
    num    = obs ? a2 : b2,       den  = obs ? a1 : b1
    a_i    = alpha_i + s_prev,    b_i  = beta_i + f_prev
    ab_i   = alpha_i + beta_i
  with s_prev/f_prev the shifted cumulative success/failure counts.

v2 engine mapping (B=4096 rows split 512/core across 8 cores; rows on
SBUF partitions, T on the free dim, F=2048 t-chunks, rc-inner order):
  DVE : a1-scan, d-scan, den select (copy_predicated in place on the
        PSUM b1), small carry copies.
  PE  : u = ab1 + t into PSUM bank A (affine matmul from [ab1; ones]
        weights x [ones; iota] rows), b1 = u - a1 into PSUM bank B
        (same affine then accumulate (-I) @ a1).  fp32 matmuls.
  GPS : numP = obs*dd + den (stt), num2 = (numP + dbe) * u (stt),
        den2 = (iota + ab2t) * den (stt).
  ACT : a2 = a1 + dal (bf16 out), b2 = b1 + dbe (bf16), b1 -> bf16,
        Ln(num2), Ln(den2), post = Sigmoid(-d - c0) (bf16).  Uses two
        activation tables (ln/exp + sigmoid); posts are batched per
        t-chunk across the 4 row-chunks so the table switch cost is
        2 loads per t-chunk.
Outputs: a1 fp32, b1/a2/b2/post bf16 (host upcasts to fp32).
"""

import numpy as np

B, T = 4096, 8192
NCORES = 8
RPC = B // NCORES        # rows per core = 512
P = 128                  # SBUF partitions
RC_N = RPC // P          # row chunks per core = 4
F = 2048                 # t-chunk width
TC_N = T // F            # t chunks = 4
# al1, dal, dbe, dd, then (ab1+t0, ab2+t0) per chunk
NCONST = 4 + 2 * TC_N

_PROGRAM_CACHE = {}


def _patch_act_tables():
    """Restrict activation-table selection to natural_log_exp_and_others
    + sigmoid_and_others (keeps dict order so act_func_set_id stays valid)."""
    import concourse.bacc as bacc_mod
    import concourse.hw_specs as hw_specs
    if getattr(bacc_mod, "_act_tables_patched", False):
        return
    orig = hw_specs.get_activation_tables

    def filtered(arch):
        full = orig(arch)
        return {
            name: (funcs if name == "natural_log_exp_and_others" else set())
            for name, funcs in full.items()
        }

    bacc_mod.get_activation_tables = filtered
    bacc_mod._act_tables_patched = True


def _build_program(c0: float):
    import concourse.bacc as bacc
    import concourse.mybir as mybir
    from concourse.tile import TileContext

    _patch_act_tables()

    f32 = mybir.dt.float32
    bf16 = mybir.dt.bfloat16
    Alu = mybir.AluOpType
    Act = mybir.ActivationFunctionType

    nc = bacc.Bacc()
    obs_d = nc.dram_tensor("obs", [RPC, T], f32, kind="ExternalInput")
    rcst_d = nc.dram_tensor("rowconst", [RPC, NCONST], f32, kind="ExternalInput")
    wcst_d = nc.dram_tensor("wconst", [4, RPC], bf16, kind="ExternalInput")
    negi_d = nc.dram_tensor("negI", [P, P], f32, kind="ExternalInput")
    posi_d = nc.dram_tensor("posI", [P, P], f32, kind="ExternalInput")
    rmat_d = nc.dram_tensor("rmat", [4, T], bf16, kind="ExternalInput")
    a1_o = nc.dram_tensor("a1_out", [RPC, T], f32, kind="ExternalOutput")
    b1_o = nc.dram_tensor("b1_out", [RPC, T], f32, kind="ExternalOutput")
    a2_o = nc.dram_tensor("a2_out", [RPC, T], bf16, kind="ExternalOutput")
    b2_o = nc.dram_tensor("b2_out", [RPC, T], bf16, kind="ExternalOutput")
    pm_o = nc.dram_tensor("post_out", [RPC, T], bf16, kind="ExternalOutput")

    with TileContext(nc) as tc:
        with (
            tc.tile_pool(name="consts", bufs=1) as cpool,
            tc.tile_pool(name="a1p", bufs=2) as a1pool,
            tc.tile_pool(name="dp", bufs=2) as dpool,
            tc.tile_pool(name="work", bufs=2) as wpool,
            tc.tile_pool(name="psum", bufs=2, space="PSUM") as ppool,
        ):
            iota_t = cpool.tile([P, F], f32, tag="iota")
            nc.gpsimd.iota(
                iota_t[:], pattern=[[1, F]], base=0, channel_multiplier=0,
                allow_small_or_imprecise_dtypes=True,
            )

            c0n_t = cpool.tile([P, 1], f32, tag="c0n")
            nc.vector.memset(c0n_t[:], c0)
            negi_t = cpool.tile([P, P], f32, tag="negI")
            nc.sync.dma_start(negi_t[:], negi_d[:, :])
            posi_t = cpool.tile([P, P], f32, tag="posI")
            nc.sync.dma_start(posi_t[:], posi_d[:, :])
            wcst_t = cpool.tile([4, RPC], bf16, tag="wconst")
            nc.sync.dma_start(wcst_t[:], wcst_d[:, :])

            rows = []
            for rc in range(RC_N):
                r0 = rc * P
                rt = cpool.tile([P, NCONST], f32, tag=f"rows{rc}")
                nc.sync.dma_start(rt[:], rcst_d[r0:r0 + P, :])
                rows.append(rt)
            carry_a1 = [cpool.tile([P, 1], f32, tag=f"ca1_{rc}", name=f"ca1_{rc}")
                        for rc in range(RC_N)]
            carry_d = [cpool.tile([P, 1], f32, tag=f"cd_{rc}", name=f"cd_{rc}")
                       for rc in range(RC_N)]

            chunks = [(tci, rc) for tci in range(TC_N) for rc in range(RC_N)]
            NCH = len(chunks)
            obs_tiles = {}
            r_tiles = {}
            st1 = {}   # k -> (a1_t, b_ps, obs_t)
            st2 = {}   # k -> (num_t, den2_t)

            def fetch_obs(k):
                tci, rc = chunks[k]
                t0, r0 = tci * F, rc * P
                obs_t = wpool.tile([P, F], f32, tag="obs", bufs=3, name="obs")
                nc.sync.dma_start(obs_t[:], obs_d[r0:r0 + P, t0:t0 + F])
                obs_tiles[k] = obs_t

            def get_r(tci):
                if tci not in r_tiles:
                    t0 = tci * F
                    r_t = wpool.tile([4, F], bf16, tag="R", bufs=1, name="r_t")
                    nc.sync.dma_start(r_t[:], rmat_d[:, t0:t0 + F])
                    r_tiles[tci] = r_t
                return r_tiles[tci]

            def emit_x1(k):
                """obs prefetch, a1 scan, PE b1 matmuls."""
                tci, rc = chunks[k]
                r0 = rc * P
                if k + 1 < NCH:
                    fetch_obs(k + 1)
                obs_t = obs_tiles[k]
                rt = rows[rc]
                al1 = rt[:, 0:1]

                a1_t = a1pool.tile([P, F + 1], f32, tag="a1", bufs=3, name="a1")
                a1_init = al1 if tci == 0 else carry_a1[rc][:]
                nc.gpsimd.tensor_copy(a1_t[:, 0:1], a1_init)
                nc.vector.tensor_tensor_scan(
                    a1_t[:, 1:F + 1], obs_t[:], obs_t[:], a1_init,
                    Alu.add, Alu.bypass,
                )
                nc.vector.tensor_copy(carry_a1[rc][:], a1_t[:, F:F + 1])
                a1_v = a1_t[:, 0:F]

                # PE: b1 = (ab1 + t) - a1 into PSUM
                b_ps = ppool.tile([P, F], f32, tag="b1", bufs=1, name="b_ps")
                wsl = wcst_t[:, r0:r0 + P]
                r_t = get_r(tci)
                for q in range(F // 512):
                    sl = slice(q * 512, (q + 1) * 512)
                    rsl = r_t[:, q * 512:(q + 1) * 512]
                    nc.tensor.matmul(
                        b_ps[:, sl], wsl, rsl, start=True, stop=False)
                    nc.tensor.matmul(
                        b_ps[:, sl], negi_t[:], a1_v[:, sl],
                        start=False, stop=True)
                st1[k] = (a1_t, b_ps, obs_t)

            def emit_x2(k):
                """b1->SBUF, output casts + DMA, select, GPS num, den2."""
                tci, rc = chunks[k]
                t0, r0 = tci * F, rc * P
                a1_t, b_ps, obs_t = st1.pop(k)
                a1_v = a1_t[:, 0:F]
                rt = rows[rc]
                dal = rt[:, 1:2]
                dbe = rt[:, 2:3]
                dd = rt[:, 3:4]
                ab2t = rt[:, 5 + 2 * tci:6 + 2 * tci]

                b1_t = wpool.tile([P, F], f32, tag="b1sb", name="b1_t")
                nc.scalar.activation(b1_t[:], b_ps[:], Act.Identity)
                a2_t = wpool.tile([P, F], bf16, tag="a2", name="a2_t")
                nc.scalar.activation(a2_t[:], a1_v, Act.Identity, bias=dal)
                b2_t = wpool.tile([P, F], bf16, tag="b2", name="b2_t")
                nc.scalar.activation(b2_t[:], b1_t[:], Act.Identity, bias=dbe)
                nc.sync.dma_start(a1_o[r0:r0 + P, t0:t0 + F], a1_v)
                nc.sync.dma_start(b1_o[r0:r0 + P, t0:t0 + F], b1_t[:])
                nc.gpsimd.dma_start(a2_o[r0:r0 + P, t0:t0 + F], a2_t[:])
                nc.gpsimd.dma_start(b2_o[r0:r0 + P, t0:t0 + F], b2_t[:])

                # den = obs ? a1 : b1  (in place in SBUF)
                obs_mask = obs_t[:].bitcast(mybir.dt.uint32)
                nc.vector.copy_predicated(b1_t[:], obs_mask, a1_v)

                # seld = obs*dd + dbe (ACT); num = den + seld (PE, into PSUM)
                # den2 = (iota + ab2t) * den (DVE stt)
                seld_t = wpool.tile([P, F], f32, tag="seld", bufs=2,
                                    name="seld_t")
                nc.scalar.activation(
                    seld_t[:], obs_t[:], Act.Identity, bias=dbe, scale=dd)
                num_ps = ppool.tile([P, F], f32, tag="num", bufs=1,
                                    name="num_ps")
                for q in range(F // 512):
                    sl = slice(q * 512, (q + 1) * 512)
                    nc.tensor.matmul(
                        num_ps[:, sl], posi_t[:], b1_t[:, sl],
                        start=True, stop=False)
                    nc.tensor.matmul(
                        num_ps[:, sl], posi_t[:], seld_t[:, sl],
                        start=False, stop=True)
                den2_t = wpool.tile([P, F], f32, tag="den2", bufs=3,
                                    name="den2_t")
                nc.vector.scalar_tensor_tensor(
                    den2_t[:], iota_t[:], ab2t, b1_t[:], Alu.add, Alu.mult)
                st2[k] = (num_ps, den2_t)

            d_done = {}

            def emit_y(k):
                """num2 stt, logs, d-scan."""
                tci, rc = chunks[k]
                num_ps, den2_t = st2.pop(k)
                rt = rows[rc]
                ab1t = rt[:, 4 + 2 * tci:5 + 2 * tci]
                num2_t = wpool.tile([P, F], f32, tag="num2", name="num2_t")
                nc.vector.scalar_tensor_tensor(
                    num2_t[:], iota_t[:], ab1t, num_ps[:], Alu.add, Alu.mult)
                # logs in place
                nc.scalar.activation(num2_t[:], num2_t[:], Act.Ln)
                nc.scalar.activation(den2_t[:], den2_t[:], Act.Ln)

                # d: state = (lnum2 + state) - lden2, chained
                d_t = dpool.tile([P, F + 1], f32, tag="d", bufs=3, name="d_t")
                if tci == 0:
                    nc.gpsimd.memset(d_t[:, 0:1], 0.0)
                    d_init = 0.0
                else:
                    d_init = carry_d[rc][:]
                    nc.gpsimd.tensor_copy(d_t[:, 0:1], d_init)
                nc.vector.tensor_tensor_scan(
                    d_t[:, 1:F + 1], num2_t[:], den2_t[:], d_init,
                    Alu.add, Alu.subtract,
                )
                nc.vector.tensor_copy(carry_d[rc][:], d_t[:, F:F + 1])
                d_done[k] = d_t

            def emit_z(k):
                """post = exp(-ln(1 + exp(d + c0))) -- single act table."""
                tci, rc = chunks[k]
                t0, r0 = tci * F, rc * P
                d_t = d_done.pop(k)
                u_t = wpool.tile([P, F], f32, tag="postu", bufs=1, name="u_t")
                nc.scalar.activation(
                    u_t[:], d_t[:, 0:F], Act.Exp, bias=c0n_t[:])
                nc.scalar.activation(u_t[:], u_t[:], Act.Ln, bias=1.0)
                post_t = wpool.tile([P, F], bf16, tag="post", name="post_t")
                nc.scalar.activation(post_t[:], u_t[:], Act.Exp, scale=-1.0)
                nc.sync.dma_start(pm_o[r0:r0 + P, t0:t0 + F], post_t[:])

            # flat 4-stage software pipeline over all 16 chunks
            # (Y before X2 so the num PSUM bank recycles cleanly)
            fetch_obs(0)
            for k in range(NCH + 3):
                if k < NCH:
                    emit_x1(k)
                if 2 <= k and k - 2 < NCH:
                    emit_y(k - 2)
                if 1 <= k and k - 1 < NCH:
                    emit_x2(k - 1)
                if 3 <= k:
                    emit_z(k - 3)
    nc.finalize()
    return nc


def _pack_inputs(alpha1, beta1, alpha2, beta2):
    """rowconst [B, NCONST]: al1, dal, dbe, dd, then ab2+t0 per chunk.
    wconst [2, B]: row0 = ab1, row1 = ones."""
    a1 = alpha1.astype(np.float32)
    b1 = beta1.astype(np.float32)
    a2 = alpha2.astype(np.float32)
    b2 = beta2.astype(np.float32)
    dal = a2 - a1
    dbe = b2 - b1
    cols = [a1, dal, dbe, dal - dbe]
    ab1 = a1 + b1
    ab2 = a2 + b2
    for tci in range(TC_N):
        cols.append(ab1 + np.float32(tci * F))
        cols.append(ab2 + np.float32(tci * F))
    rowconst = np.ascontiguousarray(np.stack(cols, axis=1), dtype=np.float32)
    import ml_dtypes
    bf = np.dtype(ml_dtypes.bfloat16)
    ab1_hi = (a1 + b1).astype(bf).astype(np.float32)
    ab1_lo = (a1 + b1) - ab1_hi
    ones = np.ones_like(ab1_hi)
    wconst = np.ascontiguousarray(
        np.stack([ab1_hi, ab1_lo, ones, ones], axis=0).astype(bf))
    return rowconst, wconst


def make_in_maps(obs_seq, alpha1, beta1, alpha2, beta2):
    obs_seq = np.ascontiguousarray(obs_seq, dtype=np.float32)
    rowconst, wconst = _pack_inputs(
        np.asarray(alpha1), np.asarray(beta1),
        np.asarray(alpha2), np.asarray(beta2),
    )
    import ml_dtypes
    bf = np.dtype(ml_dtypes.bfloat16)
    negI = np.ascontiguousarray(-np.eye(P, dtype=np.float32))
    posI = np.ascontiguousarray(np.eye(P, dtype=np.float32))
    g = np.arange(T, dtype=np.float32)
    g_hi = np.floor(g / 32.0) * 32.0
    g_lo = g - g_hi
    onesT = np.ones(T, np.float32)
    rmat = np.ascontiguousarray(
        np.stack([onesT, onesT, g_hi, g_lo], axis=0).astype(bf))
    in_maps = []
    for c in range(NCORES):
        r0 = c * RPC
        in_maps.append({
            "obs": obs_seq[r0:r0 + RPC],
            "rowconst": rowconst[r0:r0 + RPC],
            "wconst": np.ascontiguousarray(wconst[:, r0:r0 + RPC]),
            "negI": negI,
            "posI": posI,
            "rmat": rmat,
        })
    return in_maps


def kernel(obs_seq, alpha1, beta1, alpha2, beta2, mixweight):
    from concourse.bass_utils import run_bass_kernel_spmd

    w = float(np.float32(mixweight))
    c0 = float(np.float32(np.log((1.0 - w) / w)))
    key = c0
    if key not in _PROGRAM_CACHE:
        _PROGRAM_CACHE[key] = _build_program(c0)
    nc = _PROGRAM_CACHE[key]

    in_maps = make_in_maps(obs_seq, alpha1, beta1, alpha2, beta2)
    res = run_bass_kernel_spmd(nc, in_maps, core_ids=list(range(NCORES)))
    out = np.empty((5, B, T), np.float32)
    names = ["a1_out", "b1_out", "a2_out", "b2_out", "post_out"]
    for c in range(NCORES):
        r0 = c * RPC
        for k, name in enumerate(names):
            out[k, r0:r0 + RPC] = np.asarray(res.results[c][name]).astype(np.float32)
    return out
